# revision 12
# baseline (speedup 1.0000x reference)
"""Trainium2 Bass kernel for margin-ranking + weighted-BCE loss pair.

Math
----
reference:
  margin_loss = sum_{i<j}[ (m - dp*dl) if dp*dl < m else 0 ] / B
              = sum_{i<j} relu(m - prod_ij) / B
  with prod_ij = (p_i - p_j)(l_i - l_j) symmetric in (i,j) and prod_ii = 0:
  S_full := sum_{i,j in [B]^2} relu(m - prod_ij) = 2*S_upper + B*relu(m)
  => margin_loss = S_full/(2B) - relu(m)/2

  M_ij := m - prod_ij = p_i*l_j + l_i*p_j + 1*(m - u_j) + u_i*(-1),  u = p*l
  i.e. a rank-4 outer product -> one K=4 matmul materializes any tile of M.

Distribution: the 16x16 grid of 512x512 blocks of M, keeping only the upper
triangle (136 blocks, computed once, off-diag weighted 2x / diag 1x via a
0.5 scale on diag + global 2x folded into the formula). Core c gets the two
row-bands {c, 15-c} -> always exactly 17 blocks (2 diagonal + 15 off-diag),
so one uniform SPMD program serves all 8 cores; the host feeds each core its
own gathered row/col slices.

Per block: 4 fp32[r] matmuls (K=4, M=128, N=512) -> 4 PSUM banks, then one
fused relu+accumulate instruction over [128, 2048] on ScalarE (9 blocks) or
VectorE (8 blocks), so the two elementwise engines split the reduction work.
BCE runs on a 1024-element shard per core with the reference's exact
exp/log formulation. A final ones-matmul reduces partitions; the host sums
the 8 [margin_partial, bce_partial] pairs and applies the closed-form
corrections.
"""

import numpy as np

import concourse.bacc as bacc
import concourse.bass as bass
import concourse.mybir as mybir
import concourse.tile as tile
from concourse.bass_utils import run_bass_kernel_spmd

B = 8192
NCORES = 8
SBLK = 512                 # pairwise block side
NBANDS = B // SBLK         # 16
T = 17                     # blocks per core
FL = T * SBLK              # 8704 flattened row/col elements per core
P = 128
FREE = FL // P             # 68
BCE_N = B // NCORES        # 1024 -> [128, 8]
BCE_F = BCE_N // P         # 8

# blocks whose relu+reduce runs on ScalarE (rest on VectorE).
# t=0,1 are the diagonal blocks and need the 0.5 pre-scale that only the
# activation instruction provides.
ACT_TS = (0, 1, 2, 4, 6, 8, 10, 12, 14)

f32 = mybir.dt.float32
f32r = mybir.dt.float32r


def _block_schedule(core: int):
    """17 (row_band, col_band) pairs for `core`; diagonal blocks first."""
    bands = (core, NBANDS - 1 - core)
    blocks = [(bands[0], bands[0]), (bands[1], bands[1])]
    for r in bands:
        for cb in range(r, NBANDS):
            if cb != r:
                blocks.append((r, cb))
    assert len(blocks) == T
    return blocks


def _build_program(margin: float, mm_f32r: bool, skip: tuple = ()):
    from contextlib import ExitStack

    nc = bacc.Bacc("TRN2", target_bir_lowering=False, debug=False,
                   num_devices=NCORES)
    Relu = mybir.ActivationFunctionType.Relu
    Exp = mybir.ActivationFunctionType.Exp
    Ln = mybir.ActivationFunctionType.Ln
    add = mybir.AluOpType.add
    mult = mybir.AluOpType.mult
    amax = mybir.AluOpType.max

    rowp_d = nc.dram_tensor("rowp", [P, FREE], f32, kind="ExternalInput")
    rowl_d = nc.dram_tensor("rowl", [P, FREE], f32, kind="ExternalInput")
    colp_d = nc.dram_tensor("colp", [P, FREE], f32, kind="ExternalInput")
    coll_d = nc.dram_tensor("coll", [P, FREE], f32, kind="ExternalInput")
    blg_d = nc.dram_tensor("blg", [P, BCE_F], f32, kind="ExternalInput")
    btg_d = nc.dram_tensor("btg", [P, BCE_F], f32, kind="ExternalInput")
    pw_d = nc.dram_tensor("pw", [P, 1], f32, kind="ExternalInput")
    out_d = nc.dram_tensor("out", [1, 2], f32, kind="ExternalOutput")

    with tile.TileContext(nc) as tc, ExitStack() as ctx:
        big = ctx.enter_context(tc.tile_pool(name="big", bufs=1))
        small = ctx.enter_context(tc.tile_pool(name="small", bufs=1))
        scr = ctx.enter_context(tc.tile_pool(name="scr", bufs=2))
        psum = ctx.enter_context(
            tc.tile_pool(name="psum", bufs=2, space=bass.MemorySpace.PSUM))

        # ---- assemble the K=4 operand planes ----------------------------
        # lhs_all rows: [p_row, l_row, 1, u_row]; rhs_all: [l_col, p_col,
        # m - u_col, -1]; each row is the 17-block flattened vector.
        # For f32r matmuls, walrus requires every operand byte to come from
        # an instruction that *rounds* to f32r, so the planes are fed from
        # DVE-written f32r [128, FREE] tiles via SBUF->SBUF DMAs.
        mdt = f32r if mm_f32r else f32
        lhs_all = big.tile([4, FL], mdt, tag="lhs")
        rhs_all = big.tile([4, FL], mdt, tag="rhs")

        rowp2 = small.tile([P, FREE], f32, tag="rowp2")
        rowl2 = small.tile([P, FREE], f32, tag="rowl2")
        colp2 = small.tile([P, FREE], f32, tag="colp2")
        coll2 = small.tile([P, FREE], f32, tag="coll2")
        nc.sync.dma_start(out=rowp2[:, :], in_=rowp_d[:, :])
        nc.sync.dma_start(out=rowl2[:, :], in_=rowl_d[:, :])
        nc.sync.dma_start(out=colp2[:, :], in_=colp_d[:, :])
        nc.sync.dma_start(out=coll2[:, :], in_=coll_d[:, :])

        u_row = small.tile([P, FREE], mdt, tag="u_row")
        mu_col = small.tile([P, FREE], f32, tag="mu_col")
        mu_colr = small.tile([P, FREE], mdt, tag="mu_colr")
        nc.vector.tensor_mul(u_row[:, :], rowp2[:, :], rowl2[:, :])
        nc.vector.tensor_mul(mu_col[:, :], colp2[:, :], coll2[:, :])
        # mu = -u_col + m
        nc.vector.tensor_scalar(mu_colr[:, :], mu_col[:, :], -1.0,
                                float(margin), mult, add)
        nc.sync.dma_start(out=lhs_all[3:4, :], in_=u_row[:, :])
        nc.sync.dma_start(out=rhs_all[2:3, :], in_=mu_colr[:, :])

        if mm_f32r:
            rowp2r = small.tile([P, FREE], mdt, tag="rowp2r")
            rowl2r = small.tile([P, FREE], mdt, tag="rowl2r")
            colp2r = small.tile([P, FREE], mdt, tag="colp2r")
            coll2r = small.tile([P, FREE], mdt, tag="coll2r")
            nc.vector.tensor_copy(rowp2r[:, :], rowp2[:, :])
            nc.vector.tensor_copy(rowl2r[:, :], rowl2[:, :])
            nc.vector.tensor_copy(colp2r[:, :], colp2[:, :])
            nc.vector.tensor_copy(coll2r[:, :], coll2[:, :])
        else:
            rowp2r, rowl2r, colp2r, coll2r = rowp2, rowl2, colp2, coll2
        nc.sync.dma_start(out=lhs_all[0:1, :], in_=rowp2r[:, :])
        nc.sync.dma_start(out=lhs_all[1:2, :], in_=rowl2r[:, :])
        nc.sync.dma_start(out=rhs_all[0:1, :], in_=coll2r[:, :])
        nc.sync.dma_start(out=rhs_all[1:2, :], in_=colp2r[:, :])

        onesf = small.tile([P, FREE], mdt, tag="onesf")
        negf = small.tile([P, FREE], mdt, tag="negf")
        nc.vector.tensor_scalar(onesf[:, :], rowp2[:, :], 0.0, 1.0, mult, add)
        nc.vector.tensor_scalar(negf[:, :], rowp2[:, :], 0.0, -1.0, mult, add)
        nc.sync.dma_start(out=lhs_all[2:3, :], in_=onesf[:, :])
        nc.sync.dma_start(out=rhs_all[3:4, :], in_=negf[:, :])

        # ---- BCE on the 1024-element shard (emitted first so ScalarE's
        # exp/ln table set loads before the relu stream) ------------------
        zt = small.tile([P, BCE_F], f32, tag="zt")
        tt = small.tile([P, BCE_F], f32, tag="tt")
        pwt = small.tile([P, 1], f32, tag="pwt")
        nc.sync.dma_start(out=zt[:, :], in_=blg_d[:, :])
        nc.sync.dma_start(out=tt[:, :], in_=btg_d[:, :])
        nc.sync.dma_start(out=pwt[:, :], in_=pw_d[:, :])

        mv = small.tile([P, BCE_F], f32, tag="mv")
        zm = small.tile([P, BCE_F], f32, tag="zm")
        e1 = small.tile([P, BCE_F], f32, tag="e1")
        e2 = small.tile([P, BCE_F], f32, tag="e2")
        esum = small.tile([P, BCE_F], f32, tag="esum")
        lg = small.tile([P, BCE_F], f32, tag="lgv")
        so = small.tile([P, BCE_F], f32, tag="so")
        wv = small.tile([P, BCE_F], f32, tag="wv")
        r1 = small.tile([P, BCE_F], f32, tag="r1")
        tz = small.tile([P, BCE_F], f32, tag="tz")
        r2 = small.tile([P, BCE_F], f32, tag="r2")
        pwm1 = small.tile([P, 1], f32, tag="pwm1")
        bce_el = small.tile([P, BCE_F], f32, tag="bce_el")
        bce_acc = small.tile([P, 1], f32, tag="bce_acc")

        if "bce" in skip:
            nc.vector.memset(bce_acc[:, :], 0.0)
        else:
            # mv = relu(-z) = max(-z, 0)
            nc.vector.tensor_scalar_mul(mv[:, :], zt[:, :], -1.0)
            nc.vector.tensor_scalar_max(mv[:, :], mv[:, :], 0.0)
            nc.vector.tensor_add(zm[:, :], zt[:, :], mv[:, :])
            nc.scalar.activation(e1[:, :], mv[:, :], Exp, scale=-1.0)
            nc.scalar.activation(e2[:, :], zm[:, :], Exp, scale=-1.0)
            nc.vector.tensor_add(esum[:, :], e1[:, :], e2[:, :])
            nc.scalar.activation(lg[:, :], esum[:, :], Ln)
            nc.vector.tensor_add(so[:, :], lg[:, :], mv[:, :])
            nc.vector.tensor_scalar_add(pwm1[:, :], pwt[:, :], -1.0)
            nc.vector.tensor_scalar(wv[:, :], tt[:, :], pwm1[:, 0:1], 1.0,
                                    mult, add)
            nc.vector.tensor_mul(r1[:, :], wv[:, :], so[:, :])
            nc.vector.tensor_mul(tz[:, :], tt[:, :], zt[:, :])
            nc.vector.tensor_sub(r2[:, :], zt[:, :], tz[:, :])
            nc.vector.tensor_add(bce_el[:, :], r1[:, :], r2[:, :])
            nc.vector.tensor_reduce(bce_acc[:, :], bce_el[:, :],
                                    axis=mybir.AxisListType.X, op=add)

        # ---- the 17 pairwise blocks -------------------------------------
        n_act = len(ACT_TS)
        n_dve = T - n_act
        acc_a = small.tile([P, n_act], f32, tag="acc_a")
        acc_d = small.tile([P, n_dve], f32, tag="acc_d")

        lhs_mm = lhs_all[:, :]
        rhs_mm = rhs_all[:, :]

        ia = 0
        idv = 0
        for t in range(T):
            pb = psum.tile([P, 4, SBLK], f32, tag="blk")
            for q in range(4):
                nc.tensor.matmul(
                    pb[:, q, :],
                    lhs_mm[:, SBLK * t + P * q: SBLK * t + P * (q + 1)],
                    rhs_mm[:, SBLK * t: SBLK * (t + 1)],
                    start=True, stop=True,
                )
            if t in ACT_TS:
                s = scr.tile([P, 4, SBLK], f32, tag="scr_a")
                nc.scalar.activation(s[:, :, :], pb[:, :, :], Relu,
                                     scale=(0.5 if t < 2 else 1.0),
                                     accum_out=acc_a[:, ia: ia + 1])
                ia += 1
            else:
                s = scr.tile([P, 4, SBLK], f32, tag="scr_d")
                nc.vector.tensor_scalar(s[:, :, :], pb[:, :, :], 0.0, 0.0,
                                        amax, add,
                                        accum_out=acc_d[:, idv: idv + 1])
                idv += 1
        assert ia == n_act and idv == n_dve

        # ---- final reduction --------------------------------------------
        red_a = small.tile([P, 1], f32, tag="red_a")
        red_d = small.tile([P, 1], f32, tag="red_d")
        stacked = small.tile([P, 2], f32, tag="stacked")
        nc.vector.tensor_reduce(red_a[:, :], acc_a[:, :],
                                axis=mybir.AxisListType.X, op=add)
        nc.vector.tensor_reduce(red_d[:, :], acc_d[:, :],
                                axis=mybir.AxisListType.X, op=add)
        nc.vector.tensor_add(stacked[:, 0:1], red_a[:, :], red_d[:, :])
        nc.vector.tensor_copy(stacked[:, 1:2], bce_acc[:, :])

        if "final" in skip:
            nc.sync.dma_start(out=out_d[:, :], in_=stacked[0:1, 0:2])
        else:
            ones1 = small.tile([P, 1], f32, tag="ones1")
            nc.vector.memset(ones1[:, :], 1.0)
            pfin = psum.tile([1, 2], f32, tag="blk")
            nc.tensor.matmul(pfin[:, :], ones1[:, :], stacked[:, :],
                             start=True, stop=True)
            outt = small.tile([1, 2], f32, tag="outt")
            nc.scalar.copy(outt[:, :], pfin[:, :])
            nc.sync.dma_start(out=out_d[:, :], in_=outt[:, :])

    nc.compile()
    return nc


_programs: dict = {}


def _get_program(margin: float, mm_f32r: bool):
    key = (margin, mm_f32r)
    if key not in _programs:
        _programs[key] = _build_program(margin, mm_f32r)
    return _programs[key]


def _make_in_maps(preds, labels, logits, targets, pos_weight):
    p = np.ascontiguousarray(np.asarray(preds, np.float32))
    l = np.ascontiguousarray(np.asarray(labels, np.float32))
    z = np.ascontiguousarray(np.asarray(logits, np.float32))
    tg = np.ascontiguousarray(np.asarray(targets, np.float32))
    pw = float(np.asarray(pos_weight, np.float32).reshape(-1)[0])
    in_maps = []
    for c in range(NCORES):
        blocks = _block_schedule(c)
        rowp = np.concatenate([p[SBLK * r: SBLK * (r + 1)] for r, _ in blocks])
        rowl = np.concatenate([l[SBLK * r: SBLK * (r + 1)] for r, _ in blocks])
        colp = np.concatenate([p[SBLK * cb: SBLK * (cb + 1)] for _, cb in blocks])
        coll = np.concatenate([l[SBLK * cb: SBLK * (cb + 1)] for _, cb in blocks])
        in_maps.append({
            "rowp": rowp.reshape(P, FREE),
            "rowl": rowl.reshape(P, FREE),
            "colp": colp.reshape(P, FREE),
            "coll": coll.reshape(P, FREE),
            "blg": z[BCE_N * c: BCE_N * (c + 1)].reshape(P, BCE_F).copy(),
            "btg": tg[BCE_N * c: BCE_N * (c + 1)].reshape(P, BCE_F).copy(),
            "pw": np.full((P, 1), pw, np.float32),
        })
    return in_maps


def _combine(outs: np.ndarray, margin: float) -> np.ndarray:
    # outs: [NCORES, 1, 2] per-core partials
    s_half = float(outs[:, 0, 0].sum())
    s_bce = float(outs[:, 0, 1].sum())
    margin_loss = s_half / B - max(float(margin), 0.0) / 2.0
    bce_loss = s_bce / B
    return np.array([margin_loss, bce_loss], dtype=np.float32)


MM_F32R = True


def _run(inputs: dict, trace: bool = False, mm_f32r: bool | None = None,
         **spmd_kwargs):
    if mm_f32r is None:
        mm_f32r = MM_F32R
    m = float(np.asarray(inputs["margin"]))
    nc = _get_program(m, mm_f32r)
    in_maps = _make_in_maps(inputs["preds"], inputs["labels"],
                            inputs["logits"], inputs["targets"],
                            inputs["pos_weight"])
    res = run_bass_kernel_spmd(nc, in_maps, core_ids=list(range(NCORES)),
                               trace=trace, **spmd_kwargs)
    outs = np.stack([r["out"] for r in res.results])
    return _combine(outs, m), res


def kernel(preds, labels, logits, targets, pos_weight, margin):
    out, _ = _run(dict(preds=preds, labels=labels, logits=logits,
                       targets=targets, pos_weight=pos_weight,
                       margin=margin))
    return out


# revision 13
# speedup vs baseline: 1.1573x; 1.1573x over previous
"""Trainium2 Bass kernel for margin-ranking + weighted-BCE loss pair.

Math
----
reference:
  margin_loss = sum_{i<j}[ (m - dp*dl) if dp*dl < m else 0 ] / B
              = sum_{i<j} relu(m - prod_ij) / B
  with prod_ij = (p_i - p_j)(l_i - l_j) symmetric in (i,j) and prod_ii = 0:
  S_full := sum_{i,j in [B]^2} relu(m - prod_ij) = 2*S_upper + B*relu(m)
  => margin_loss = S_full/(2B) - relu(m)/2

  M_ij := m - prod_ij = p_i*l_j + l_i*p_j + 1*(m - u_j) + u_i*(-1),  u = p*l
  i.e. a rank-4 outer product -> one K=4 matmul materializes any tile of M.

Distribution: the 16x16 grid of 512x512 blocks of M, keeping only the upper
triangle (136 blocks, computed once, off-diag weighted 2x / diag 1x via a
0.5 scale on the diagonal + global 2x folded into the formula). Core c gets
row-bands {c, 15-c} -> always exactly 17 blocks (2 diagonal + 15 off-diag),
so one uniform SPMD program serves all 8 cores; the host feeds each core its
own gathered row/col slices (pure slicing/layout, no arithmetic).

Per block: 4 bf16 matmuls (K=4, M=128, N=512) packed 4-wide onto the PE
array via tile_position row groups (operand planes replicated at partitions
0/32/64/96), filling 4 PSUM banks as 2 two-bank tiles. Each [128, 1024]
PSUM chunk is consumed by ONE fused relu+accumulate instruction on ScalarE
or VectorE, so the two elementwise engines split the reduction load. BCE
runs on a 1024-element f32 shard per core with the reference's exact
exp/log formulation. A final ones-matmul reduces partitions; the host sums
the 8 [margin_partial, bce_partial] pairs and applies closed-form
corrections.
"""

import numpy as np
import ml_dtypes

import concourse.bacc as bacc
import concourse.bass as bass
import concourse.mybir as mybir
import concourse.tile as tile
from concourse.bass_utils import run_bass_kernel_spmd

B = 8192
NCORES = 8
SBLK = 512                 # pairwise block side
NBANDS = B // SBLK         # 16
T = 17                     # blocks per core
FL = T * SBLK              # 8704 flattened row/col elements per core
P = 128
P32 = 32
F272 = FL // P32           # 272
BCE_N = B // NCORES        # 1024 -> [128, 8]
BCE_F = BCE_N // P         # 8
NCHUNK = 2 * T             # 34 half-block [128, 1024] relu chunks

# chunks whose relu+reduce runs on ScalarE (rest on VectorE). Chunks 0..3
# are the two diagonal blocks and need the 0.5 pre-scale that only the
# activation instruction provides.
ACT_H = frozenset((0, 1, 2, 3)) | frozenset(range(4, NCHUNK, 2))

f32 = mybir.dt.float32
f32r = mybir.dt.float32r
bf16 = mybir.dt.bfloat16


def _block_schedule(core: int):
    """17 (row_band, col_band) pairs for `core`; diagonal blocks first."""
    bands = (core, NBANDS - 1 - core)
    blocks = [(bands[0], bands[0]), (bands[1], bands[1])]
    for r in bands:
        for cb in range(r, NBANDS):
            if cb != r:
                blocks.append((r, cb))
    assert len(blocks) == T
    return blocks


def _build_program(margin: float, mode: str, skip: tuple = ()):
    from contextlib import ExitStack

    assert mode in ("bf16", "f32r")
    nc = bacc.Bacc("TRN2", target_bir_lowering=False, debug=False,
                   num_devices=NCORES)
    Relu = mybir.ActivationFunctionType.Relu
    Exp = mybir.ActivationFunctionType.Exp
    Ln = mybir.ActivationFunctionType.Ln
    add = mybir.AluOpType.add
    mult = mybir.AluOpType.mult
    amax = mybir.AluOpType.max

    mdt = bf16 if mode == "bf16" else f32r
    rowp_d = nc.dram_tensor("rowp", [P32, F272], mdt, kind="ExternalInput")
    rowl_d = nc.dram_tensor("rowl", [P32, F272], mdt, kind="ExternalInput")
    colp_d = nc.dram_tensor("colp", [P32, F272], mdt, kind="ExternalInput")
    coll_d = nc.dram_tensor("coll", [P32, F272], mdt, kind="ExternalInput")
    cn_d = nc.dram_tensor("cn", [2, FL], mdt, kind="ExternalInput")
    blg_d = nc.dram_tensor("blg", [P, BCE_F], f32, kind="ExternalInput")
    btg_d = nc.dram_tensor("btg", [P, BCE_F], f32, kind="ExternalInput")
    pw_d = nc.dram_tensor("pw", [P, 1], f32, kind="ExternalInput")
    out_d = nc.dram_tensor("out", [1, 2], f32, kind="ExternalOutput")

    with tile.TileContext(nc) as tc, ExitStack() as ctx:
        big = ctx.enter_context(tc.tile_pool(name="big", bufs=1))
        small = ctx.enter_context(tc.tile_pool(name="small", bufs=1))
        scr = ctx.enter_context(tc.tile_pool(name="scr", bufs=2))
        psum = ctx.enter_context(
            tc.tile_pool(name="psum", bufs=4, space=bass.MemorySpace.PSUM))

        # ---- operand planes ---------------------------------------------
        # lhs rows: [p_row, l_row, 1, u_row]; rhs rows: [l_col, p_col,
        # m - u_col, -1], replicated at partition offsets 0/32/64/96 for
        # 4-wide tile_position packing. Host supplies everything except u
        # and m-u, which are computed in [32, 272] layout and DMA-gathered.
        lhs_rep = big.tile([P, FL], mdt, tag="lhs")
        rhs_rep = big.tile([P, FL], mdt, tag="rhs")

        rp32 = small.tile([P32, F272], mdt, tag="rp32")
        rl32 = small.tile([P32, F272], mdt, tag="rl32")
        cp32 = small.tile([P32, F272], mdt, tag="cp32")
        cl32 = small.tile([P32, F272], mdt, tag="cl32")
        nc.sync.dma_start(out=rp32[:, :], in_=rowp_d[:, :])
        nc.sync.dma_start(out=rl32[:, :], in_=rowl_d[:, :])
        nc.scalar.dma_start(out=cp32[:, :], in_=colp_d[:, :])
        nc.scalar.dma_start(out=cl32[:, :], in_=coll_d[:, :])

        u16 = small.tile([P32, F272], mdt, tag="u16")
        ucol = small.tile([P32, F272], f32, tag="ucol")
        mu16 = small.tile([P32, F272], mdt, tag="mu16")
        nc.vector.tensor_mul(u16[:, :], rp32[:, :], rl32[:, :])
        nc.vector.tensor_mul(ucol[:, :], cp32[:, :], cl32[:, :])
        # mu = -u_col + m  (rounding write into the matmul dtype)
        nc.vector.tensor_scalar(mu16[:, :], ucol[:, :], -1.0,
                                float(margin), mult, add)

        nc.sync.dma_start(out=lhs_rep[0:1, :], in_=rowp_d[:, :])
        nc.sync.dma_start(out=lhs_rep[1:2, :], in_=rowl_d[:, :])
        nc.sync.dma_start(out=lhs_rep[2:3, :], in_=cn_d[0:1, :])
        nc.sync.dma_start(out=lhs_rep[3:4, :], in_=u16[:, :])
        nc.scalar.dma_start(out=rhs_rep[0:1, :], in_=coll_d[:, :])
        nc.scalar.dma_start(out=rhs_rep[1:2, :], in_=colp_d[:, :])
        nc.scalar.dma_start(out=rhs_rep[2:3, :], in_=mu16[:, :])
        nc.scalar.dma_start(out=rhs_rep[3:4, :], in_=cn_d[1:2, :])

        for q in (1, 2, 3):
            nc.sync.dma_start(out=lhs_rep[32 * q: 32 * q + 4, :],
                              in_=lhs_rep[0:4, :])
            nc.scalar.dma_start(out=rhs_rep[32 * q: 32 * q + 4, :],
                                in_=rhs_rep[0:4, :])

        # ---- BCE on the 1024-element shard (emitted first so ScalarE's
        # exp/ln table sets load during setup, before the relu stream) ----
        zt = small.tile([P, BCE_F], f32, tag="zt")
        tt = small.tile([P, BCE_F], f32, tag="tt")
        pwt = small.tile([P, 1], f32, tag="pwt")
        nc.gpsimd.dma_start(out=zt[:, :], in_=blg_d[:, :])
        nc.gpsimd.dma_start(out=tt[:, :], in_=btg_d[:, :])
        nc.gpsimd.dma_start(out=pwt[:, :], in_=pw_d[:, :])

        mv = small.tile([P, BCE_F], f32, tag="mv")
        zm = small.tile([P, BCE_F], f32, tag="zm")
        e1 = small.tile([P, BCE_F], f32, tag="e1")
        e2 = small.tile([P, BCE_F], f32, tag="e2")
        esum = small.tile([P, BCE_F], f32, tag="esum")
        lg = small.tile([P, BCE_F], f32, tag="lgv")
        so = small.tile([P, BCE_F], f32, tag="so")
        wv = small.tile([P, BCE_F], f32, tag="wv")
        r1 = small.tile([P, BCE_F], f32, tag="r1")
        tz = small.tile([P, BCE_F], f32, tag="tz")
        r2 = small.tile([P, BCE_F], f32, tag="r2")
        pwm1 = small.tile([P, 1], f32, tag="pwm1")
        bce_el = small.tile([P, BCE_F], f32, tag="bce_el")
        bce_acc = small.tile([P, 1], f32, tag="bce_acc")

        if "bce" in skip:
            nc.vector.memset(bce_acc[:, :], 0.0)
        else:
            # mv = relu(-z) = max(-z, 0)
            nc.vector.tensor_scalar_mul(mv[:, :], zt[:, :], -1.0)
            nc.vector.tensor_scalar_max(mv[:, :], mv[:, :], 0.0)
            nc.vector.tensor_add(zm[:, :], zt[:, :], mv[:, :])
            nc.scalar.activation(e1[:, :], mv[:, :], Exp, scale=-1.0)
            nc.scalar.activation(e2[:, :], zm[:, :], Exp, scale=-1.0)
            nc.vector.tensor_add(esum[:, :], e1[:, :], e2[:, :])
            nc.scalar.activation(lg[:, :], esum[:, :], Ln)
            nc.vector.tensor_add(so[:, :], lg[:, :], mv[:, :])
            nc.vector.tensor_scalar_add(pwm1[:, :], pwt[:, :], -1.0)
            nc.vector.tensor_scalar(wv[:, :], tt[:, :], pwm1[:, 0:1], 1.0,
                                    mult, add)
            nc.vector.tensor_mul(r1[:, :], wv[:, :], so[:, :])
            nc.vector.tensor_mul(tz[:, :], tt[:, :], zt[:, :])
            nc.vector.tensor_sub(r2[:, :], zt[:, :], tz[:, :])
            nc.vector.tensor_add(bce_el[:, :], r1[:, :], r2[:, :])
            nc.vector.tensor_reduce(bce_acc[:, :], bce_el[:, :],
                                    axis=mybir.AxisListType.X, op=add)

        # early, dependency-free pieces of the tail
        ones1 = small.tile([P, 1], f32, tag="ones1")
        nc.vector.memset(ones1[:, :], 1.0)

        # ---- the 17 pairwise blocks = 34 [128, 1024] relu chunks --------
        n_act = len(ACT_H)
        n_dve = NCHUNK - n_act
        acc_a = small.tile([P, n_act], f32, tag="acc_a")
        acc_d = small.tile([P, n_dve], f32, tag="acc_d")

        ia = 0
        idv = 0
        for t in range(T):
            for half in range(2):
                h = 2 * t + half
                pb = psum.tile([P, 2, SBLK], f32, tag="blk")
                for j in range(2):
                    q = 2 * half + j
                    nc.tensor.matmul(
                        pb[:, j, :],
                        lhs_rep[32 * q: 32 * q + 4,
                                SBLK * t + P * q: SBLK * t + P * (q + 1)],
                        rhs_rep[32 * q: 32 * q + 4,
                                SBLK * t: SBLK * (t + 1)],
                        start=True, stop=True,
                        tile_position=(32 * q, 0),
                    )
                if h in ACT_H:
                    s = scr.tile([P, 2, SBLK], f32, tag="scr_a")
                    nc.scalar.activation(s[:, :, :], pb[:, :, :], Relu,
                                         scale=(0.5 if t < 2 else 1.0),
                                         accum_out=acc_a[:, ia: ia + 1])
                    ia += 1
                else:
                    s = scr.tile([P, 2, SBLK], f32, tag="scr_d")
                    nc.vector.tensor_scalar(s[:, :, :], pb[:, :, :], 0.0, 0.0,
                                            amax, add,
                                            accum_out=acc_d[:, idv: idv + 1])
                    idv += 1
        assert ia == n_act and idv == n_dve

        # ---- final reduction --------------------------------------------
        red_a = small.tile([P, 1], f32, tag="red_a")
        red_d = small.tile([P, 1], f32, tag="red_d")
        stacked = small.tile([P, 2], f32, tag="stacked")
        nc.vector.tensor_reduce(red_a[:, :], acc_a[:, :],
                                axis=mybir.AxisListType.X, op=add)
        nc.vector.tensor_reduce(red_d[:, :], acc_d[:, :],
                                axis=mybir.AxisListType.X, op=add)
        nc.vector.tensor_add(stacked[:, 0:1], red_a[:, :], red_d[:, :])
        nc.vector.tensor_copy(stacked[:, 1:2], bce_acc[:, :])

        if "final" in skip:
            nc.sync.dma_start(out=out_d[:, :], in_=stacked[0:1, 0:2])
        else:
            pfin = psum.tile([1, 2], f32, tag="blk")
            nc.tensor.matmul(pfin[:, :], ones1[:, :], stacked[:, :],
                             start=True, stop=True)
            outt = small.tile([1, 2], f32, tag="outt")
            nc.scalar.copy(outt[:, :], pfin[:, :])
            nc.sync.dma_start(out=out_d[:, :], in_=outt[:, :])

    nc.compile()
    return nc


_programs: dict = {}


def _get_program(margin: float, mode: str, skip: tuple = ()):
    key = (margin, mode, skip)
    if key not in _programs:
        _programs[key] = _build_program(margin, mode, skip)
    return _programs[key]


def _make_in_maps(preds, labels, logits, targets, pos_weight, mode="bf16"):
    p = np.ascontiguousarray(np.asarray(preds, np.float32))
    l = np.ascontiguousarray(np.asarray(labels, np.float32))
    z = np.ascontiguousarray(np.asarray(logits, np.float32))
    tg = np.ascontiguousarray(np.asarray(targets, np.float32))
    pw = float(np.asarray(pos_weight, np.float32).reshape(-1)[0])
    ndt = ml_dtypes.bfloat16 if mode == "bf16" else np.float32
    cn = np.empty((2, FL), ndt)
    cn[0, :] = 1.0
    cn[1, :] = -1.0
    in_maps = []
    for c in range(NCORES):
        blocks = _block_schedule(c)
        rowp = np.concatenate([p[SBLK * r: SBLK * (r + 1)] for r, _ in blocks])
        rowl = np.concatenate([l[SBLK * r: SBLK * (r + 1)] for r, _ in blocks])
        colp = np.concatenate([p[SBLK * cb: SBLK * (cb + 1)] for _, cb in blocks])
        coll = np.concatenate([l[SBLK * cb: SBLK * (cb + 1)] for _, cb in blocks])
        in_maps.append({
            "rowp": rowp.astype(ndt).reshape(P32, F272),
            "rowl": rowl.astype(ndt).reshape(P32, F272),
            "colp": colp.astype(ndt).reshape(P32, F272),
            "coll": coll.astype(ndt).reshape(P32, F272),
            "cn": cn,
            "blg": z[BCE_N * c: BCE_N * (c + 1)].reshape(P, BCE_F).copy(),
            "btg": tg[BCE_N * c: BCE_N * (c + 1)].reshape(P, BCE_F).copy(),
            "pw": np.full((P, 1), pw, np.float32),
        })
    return in_maps


def _combine(outs: np.ndarray, margin: float) -> np.ndarray:
    # outs: [NCORES, 1, 2] per-core partials
    s_half = float(outs[:, 0, 0].sum())
    s_bce = float(outs[:, 0, 1].sum())
    margin_loss = s_half / B - max(float(margin), 0.0) / 2.0
    bce_loss = s_bce / B
    return np.array([margin_loss, bce_loss], dtype=np.float32)


MODE = "bf16"


def _run(inputs: dict, trace: bool = False, mode: str | None = None,
         **spmd_kwargs):
    if mode is None:
        mode = MODE
    m = float(np.asarray(inputs["margin"]))
    nc = _get_program(m, mode)
    in_maps = _make_in_maps(inputs["preds"], inputs["labels"],
                            inputs["logits"], inputs["targets"],
                            inputs["pos_weight"], mode=mode)
    res = run_bass_kernel_spmd(nc, in_maps, core_ids=list(range(NCORES)),
                               trace=trace, **spmd_kwargs)
    outs = np.stack([np.asarray(r["out"], np.float32) for r in res.results])
    return _combine(outs, m), res


def kernel(preds, labels, logits, targets, pos_weight, margin):
    out, _ = _run(dict(preds=preds, labels=labels, logits=logits,
                       targets=targets, pos_weight=pos_weight,
                       margin=margin))
    return out
